# revision 1
# baseline (speedup 1.0000x reference)
"""GATv2 dense-attention kernel for Trainium2 (8 NeuronCores, data-parallel over batch).

Math (per batch b, head h, with W=128 nodes, F=64 in-feats, OUTF=64):
  fsrc = x @ w_src.T + b_src            # [W, H*OUTF]
  fdst = x @ w_dst.T + b_dst
  e[i,j,h]  = sum_f a[h,f] * leakyrelu(fsrc[j,h,f] + fdst[i,h,f], 0.2)
  alpha     = softmax_j(e)
  out[i,f]  = mean_h sum_j alpha[i,j,h] * fsrc[j,h,f]

Decomposition used on device:
  leakyrelu(z) = 0.2*z + 0.8*relu(z)
  e[i,j,h] = 0.8*sum_hf a*relu(z) + 0.2*sS_h[j] + 0.2*sD_h[i]
  The sD term is constant along j and cancels in softmax_j, so it is dropped.
  relu(z) for one i is a single tensor_scalar/activation op on the
  [(h,f)=128, j=128] layout with the per-partition bias fdstT[:, i].
  The a-weighted (h,f)-reduction is a PE matmul with lhsT = qhat_i,
  rhs = acols (0.8*a per head column), giving ET[j, 2i:2i+2] directly in the
  transposed orientation that the softmax-normalizer (ones-matmul column sum)
  and the final alpha @ fsrc matmul want.  Scores are small (|e| < ~4) so the
  softmax max-subtraction is skipped.  The head-mean 0.5 is folded into the
  fsrc used by the final matmul.
"""

import functools
import sys

sys.path.insert(0, "/opt/trn_rl_repo")

import numpy as np

import bass_rust
import concourse.bass as bass
import concourse.mybir as mybir
import concourse.tile as tile
from concourse.bass_utils import run_bass_kernel_spmd

B, W, F = 64, 128, 64
H, OUTF = 2, 64
HO = H * OUTF  # 128
NCORES = 8
NB = B // NCORES  # batches per core
FP32 = mybir.dt.float32
F32R = mybir.dt.float32r
BF16 = mybir.dt.bfloat16

_wait_nop_counter = [0]


_WAIT_BUDGET = {}
_WAIT_BUDGET_DEFAULT = 1


def _legalize_waits(nc, nop_budget=1):
    """This container's walrus codegen rejects instructions carrying more than
    a struct-dependent number of sync waits (1 for Matmult S3_LW / Drain, 2
    for most compute structs).  Move excess semaphore waits onto same-engine
    NoOps inserted just before the offender."""
    for f in nc.m.functions:
        for blk in f.blocks:
            out = []
            changed = False
            for inst in blk.instructions:
                si = inst.sync_info
                if si is not None:
                    max_waits = _WAIT_BUDGET.get(str(inst.opcode), _WAIT_BUDGET_DEFAULT)
                    waits = list(si.on_wait)
                    movable = [w for w in waits
                               if w.sync_type == "semaphore"
                               and w.wait_mode == "sem-ge-imm"
                               and not w.wait_reg]
                    fixed = [w for w in waits if w not in movable]
                    budget = max(max_waits - len(fixed), 0)
                    if len(movable) > budget:
                        keep = movable[len(movable) - budget:] if budget else []
                        excess = movable[:len(movable) - budget] if budget else movable
                        for i in range(0, len(excess), nop_budget):
                            chunk = excess[i:i + nop_budget]
                            _wait_nop_counter[0] += 1
                            nop = bass_rust.InstNoOp(
                                name=f"legalize-wait-nop-{_wait_nop_counter[0]}",
                                ins=[], outs=[])
                            nop.engine = inst.engine
                            nop.sync_info = mybir.SyncInfo(on_wait=chunk, on_update=[])
                            out.append(nop)
                        inst.sync_info = mybir.SyncInfo(
                            on_wait=fixed + keep, on_update=list(si.on_update))
                        changed = True
                out.append(inst)
            if changed:
                blk.instructions = out


@functools.lru_cache(maxsize=1)
def _build():
    nc = bass.Bass("TRN2", target_bir_lowering=False)
    AF = mybir.ActivationFunctionType
    OP = mybir.AluOpType

    x_d = nc.dram_tensor("x", [NB, W, F], FP32, kind="ExternalInput")
    wsrc_d = nc.dram_tensor("wsrc_ext", [F + 1, HO], FP32, kind="ExternalInput")
    wdst_d = nc.dram_tensor("wdst_ext", [F + 1, HO], FP32, kind="ExternalInput")
    wsrch_d = nc.dram_tensor("wsrc_half", [F + 1, HO], FP32, kind="ExternalInput")
    acols_d = nc.dram_tensor("acols_bf", [HO, 2], BF16, kind="ExternalInput")
    acol2_d = nc.dram_tensor("acols02", [HO, 2], FP32, kind="ExternalInput")
    ident_d = nc.dram_tensor("ident", [W, W], FP32, kind="ExternalInput")
    ones_d = nc.dram_tensor("ones_col", [W, 1], FP32, kind="ExternalInput")
    out_d = nc.dram_tensor("out", [NB, W, OUTF], FP32, kind="ExternalOutput")

    with tile.TileContext(nc) as tc:
        with tc.tile_pool(name="const", bufs=1) as cpool, \
             tc.tile_pool(name="proj", bufs=3) as proj, \
             tc.tile_pool(name="qhat", bufs=2) as qpool, \
             tc.tile_pool(name="soft", bufs=3) as soft, \
             tc.tile_pool(name="pps", bufs=4, space="PSUM") as pps, \
             tc.tile_pool(name="eps", bufs=3, space="PSUM") as eps:

            wsrc = cpool.tile([F + 1, HO], FP32)
            wdst = cpool.tile([F + 1, HO], FP32)
            wsrch = cpool.tile([F + 1, HO], FP32)
            acols = cpool.tile([HO, 2], BF16)
            acol2 = cpool.tile([HO, 2], FP32)
            ident = cpool.tile([W, W], FP32)
            ones = cpool.tile([W, 1], FP32)
            nc.sync.dma_start(wsrc[:], wsrc_d[:])
            nc.sync.dma_start(wdst[:], wdst_d[:])
            nc.sync.dma_start(wsrch[:], wsrch_d[:])
            nc.sync.dma_start(acols[:], acols_d[:])
            nc.sync.dma_start(acol2[:], acol2_d[:])
            nc.sync.dma_start(ident[:], ident_d[:])
            nc.sync.dma_start(ones[:], ones_d[:])

            state = {}

            def stage_head(b):
                # ---- load + transpose x_b --------------------------------
                x_nat = proj.tile([W, F], FP32, tag="x_nat")
                nc.sync.dma_start(x_nat[:], x_d[b])
                xt_ps = pps.tile([F, W], FP32, tag="ps_scratch")
                nc.tensor.transpose(xt_ps[:], x_nat[:], ident[:])
                xt_ext = proj.tile([F + 1, W], FP32, tag="xt_ext")
                nc.vector.memset(xt_ext[F:F + 1, :], 1.0)
                nc.vector.tensor_copy(xt_ext[0:F, :], xt_ps[:])

                # ---- projections ----------------------------------------
                fsrcT_ps = pps.tile([HO, W], FP32, tag="ps_scratch")
                nc.tensor.matmul(fsrcT_ps[:], wsrc[:], xt_ext[:], start=True, stop=True)
                fsrcT = proj.tile([HO, W], FP32, tag="fsrcT")
                nc.scalar.copy(fsrcT[:], fsrcT_ps[:])
                fsrcT_bf = proj.tile([HO, W], BF16, tag="fsrcT_bf")
                nc.vector.tensor_copy(fsrcT_bf[:], fsrcT_ps[:])

                fdstT_ps = pps.tile([HO, W], FP32, tag="ps_scratch")
                nc.tensor.matmul(fdstT_ps[:], wdst[:], xt_ext[:], start=True, stop=True)
                fdstT = proj.tile([HO, W], FP32, tag="fdstT")
                nc.scalar.copy(fdstT[:], fdstT_ps[:])

                fsrcN_ps = pps.tile([W, HO], FP32, tag="ps_scratch")
                nc.tensor.matmul(fsrcN_ps[:], xt_ext[:], wsrch[:], start=True, stop=True)
                fsrcN = proj.tile([W, HO], FP32, tag="fsrcN")
                nc.scalar.copy(fsrcN[:], fsrcN_ps[:])

                # sS columns: [j, h] = 0.2 * (fsrc @ a_h)
                ss_ps = pps.tile([W, 2], FP32, tag="ps_scratch")
                nc.tensor.matmul(ss_ps[:], fsrcT[:], acol2[:], start=True, stop=True)
                ssc = proj.tile([W, 2], FP32, tag="ssc")
                nc.vector.tensor_copy(ssc[:], ss_ps[:])
                state[b] = dict(fsrcT=fsrcT, fsrcT_bf=fsrcT_bf, fdstT=fdstT,
                                fsrcN=fsrcN, ssc=ssc)

            def stage_attn(b, i_lo, i_hi):
                st = state[b]
                fsrcT, fsrcT_bf, fdstT = st["fsrcT"], st["fsrcT_bf"], st["fdstT"]
                if i_lo == 0:
                    st["ET"] = eps.tile([W, 2 * W], FP32, tag="ET", name="ET")
                    st["qbig"] = qpool.tile([HO, W * W], BF16, tag="qbig", name="qbig")
                ET, qbig = st["ET"], st["qbig"]
                for i in range(i_lo, i_hi):
                    qs = qbig[:, W * i:W * (i + 1)]
                    r = i % 32
                    if r in (0, 5, 11, 20, 22, 27):
                        nc.scalar.activation(qs, fsrcT[:], AF.Relu,
                                             bias=fdstT[:, i:i + 1], scale=1.0)
                    elif r in (2, 7, 9, 14, 18, 25, 29):
                        nc.gpsimd.tensor_scalar(
                            out=qs, in0=fsrcT_bf[:],
                            scalar1=fdstT[:, i:i + 1], scalar2=0.0,
                            op0=OP.add, op1=OP.max)
                    else:
                        nc.vector.tensor_scalar(
                            out=qs, in0=fsrcT_bf[:],
                            scalar1=fdstT[:, i:i + 1], scalar2=0.0,
                            op0=OP.add, op1=OP.max)
                    nc.tensor.matmul(ET[:, 2 * i:2 * i + 2], qbig[:, W * i:W * (i + 1)],
                                     acols[:], start=True, stop=True)

            def stage_tail(b):
                st = state[b]
                ET, ssc, fsrcN = st["ET"], st["ssc"], st["fsrcN"]
                # ---- softmax (no max-subtraction; scores are small) ------
                PT = soft.tile([W, 2 * W], FP32, tag="PT")
                for h in range(H):
                    nc.scalar.activation(
                        PT[:].rearrange("j (i h) -> j i h", h=2)[:, :, h],
                        ET[:].rearrange("j (i h) -> j i h", h=2)[:, :, h],
                        AF.Exp, bias=ssc[:, h:h + 1], scale=1.0)

                # Z^T[i, h] = sum_j PT[j, (i,h)]  (column sums via PE)
                zc_ps = pps.tile([W, 2], FP32, tag="ps_scratch")
                for h in range(H):
                    nc.tensor.matmul(
                        zc_ps[:, h:h + 1],
                        PT[:].rearrange("j (i h) -> j i h", h=2)[:, :, h],
                        ones[:], start=True, stop=True)
                rzc = soft.tile([W, 2], FP32, tag="rzc")
                nc.vector.reciprocal(rzc[:], zc_ps[:])

                # ---- final: out[i,f] = sum_h (1/Z[i,h]) sum_j PT[j,(i,h)] fsrcN[j,(h,f)]
                f0_ps = pps.tile([W, OUTF], FP32, tag="ps_scratch")
                f1_ps = pps.tile([W, OUTF], FP32, tag="ps_scratch")
                for h, fps in enumerate((f0_ps, f1_ps)):
                    nc.tensor.matmul(
                        fps[:],
                        PT[:].rearrange("j (i h) -> j i h", h=2)[:, :, h],
                        fsrcN[:, h * OUTF:(h + 1) * OUTF],
                        start=True, stop=True)
                f_sb = soft.tile([W, OUTF], FP32, tag="f_sb")
                nc.vector.tensor_scalar_mul(f_sb[:], f0_ps[:], rzc[:, 0:1])
                f_out = soft.tile([W, OUTF], FP32, tag="f_out")
                nc.vector.scalar_tensor_tensor(
                    out=f_out[:], in0=f1_ps[:], scalar=rzc[:, 1:2], in1=f_sb[:],
                    op0=OP.mult, op1=OP.add)
                nc.sync.dma_start(out_d[b], f_out[:])
                del state[b]

            # software pipeline: tail(b-1) is emitted inside b's attn phase
            PIPE_SPLIT = 64
            for b in range(NB):
                stage_head(b)
                stage_attn(b, 0, PIPE_SPLIT)
                if b > 0:
                    stage_tail(b - 1)
                stage_attn(b, PIPE_SPLIT, W)
            stage_tail(NB - 1)

    _legalize_waits(nc)
    return nc


def _make_consts(w_src, b_src, w_dst, b_dst, attn_w):
    wsrc_ext = np.concatenate([w_src.T, b_src[None, :]], axis=0).astype(np.float32)
    wdst_ext = np.concatenate([w_dst.T, b_dst[None, :]], axis=0).astype(np.float32)
    wsrc_half = (0.5 * wsrc_ext).astype(np.float32)
    acols = np.zeros((HO, 2), np.float32)
    acols[0:OUTF, 0] = 0.8 * attn_w[0]
    acols[OUTF:HO, 1] = 0.8 * attn_w[1]
    import ml_dtypes
    acols_bf = acols.astype(ml_dtypes.bfloat16)
    acol2 = (0.25 * acols).astype(np.float32)  # 0.2 * a
    ident = np.eye(W, dtype=np.float32)
    ones = np.ones((W, 1), np.float32)
    return dict(wsrc_ext=wsrc_ext, wdst_ext=wdst_ext, wsrc_half=wsrc_half,
                acols_bf=acols_bf, acols02=acol2, ident=ident, ones_col=ones)


def kernel(x, w_src, b_src, w_dst, b_dst, attn_w):
    x = np.ascontiguousarray(np.asarray(x, dtype=np.float32))
    consts = _make_consts(np.asarray(w_src, np.float32), np.asarray(b_src, np.float32),
                          np.asarray(w_dst, np.float32), np.asarray(b_dst, np.float32),
                          np.asarray(attn_w, np.float32))
    nc = _build()
    in_maps = []
    for c in range(NCORES):
        m = {"x": np.ascontiguousarray(x[c * NB:(c + 1) * NB])}
        m.update(consts)
        in_maps.append(m)
    res = run_bass_kernel_spmd(nc, in_maps, core_ids=list(range(NCORES)))
    out = np.concatenate([r["out"] for r in res.results], axis=0)
    return out.astype(np.float32)

